# revision 44
# baseline (speedup 1.0000x reference)
"""Trainium2 Bass kernel for nn_CONVMGEmbedder (3-layer GraphConv + UnitedNorm + readout).

Strategy: dst-sharded graph partition over 8 NeuronCores.
- Node shard k = rows [k*12500, (k+1)*12500), padded to 12544 (98 blocks of 128).
- Edges live on their dst-owner core, grouped by (dst block, src shard-quarter),
  padded to a global (SPMD-uniform) tile table.
- Layer 0: edge rows are pre-gathered on HOST (node_feats*inv_sqrt_out taken per
  edge) and streamed sequentially via HWDGE — no random-access gather.
- Layers 1-2: dma_gather of m[src] rows from per-quarter AllGathered tables,
  one-hot S tiles (DVE iota/is_equal), PE matmuls accumulate aggT = sum_e m_e x 1_slot.
- h = (aggT.T @ W) * inv_sqrt_in (ACT copy w/ per-node scale, fused row-sum for
  node stats). All matmul operands bf16.
- UnitedNorm: u = h*P - Q with P/Q = graph coeffs (one-hot matmul) + per-node
  coeffs (K=1 matmul) accumulated in PSUM; 16KB AllReduce for graph/batch stats.
- m_{l+1} = Lrelu(u * inv_sqrt_out) via one fused ACT op, stored per shard
  quarter; per-quarter AllGathers issue progressively during pass B.
- Readout: G^T @ h3 accumulated in PSUM, AllReduce, /cnt, leaky.
"""
import math
import os
import sys

sys.path.insert(0, "/opt/trn_rl_repo")

import numpy as np


def _cfg_real():
    return dict(
        N=100000, E=1600000, C=128, B=16, L=3, NCORES=8,
        NBUCK=4, CH=8, GD="bf16", GB=2,
    )


def _derive(cfg):
    c = dict(cfg)
    c["SHARD"] = c["N"] // c["NCORES"]
    c["NBLK"] = (c["SHARD"] + 127) // 128
    c["SHARD_PAD"] = c["NBLK"] * 128
    c["NROWS"] = c["NCORES"] * c["SHARD_PAD"]
    # split NBLK blocks into NBUCK nearly-equal chunks (shard quarters)
    base, rem = divmod(c["NBLK"], c["NBUCK"])
    c["CH_BLOCKS"] = [base] * (c["NBUCK"] - rem) + [base + 1] * rem
    c["CH_ROWS"] = [b * 128 for b in c["CH_BLOCKS"]]
    c["CB0"] = np.concatenate([[0], np.cumsum(c["CH_BLOCKS"])[:-1]]).tolist()
    c["CSTART"] = [b * 128 for b in c["CB0"]]
    c["WIN"] = [c["NCORES"] * r for r in c["CH_ROWS"]]
    assert all(w <= 32768 for w in c["WIN"]), c["WIN"]
    c["EPS"] = 1e-5
    c["SLOPE"] = 0.2
    return c


def prep_host(inputs, cfg):
    """Pure-numpy sharding prep: degrees, edge reorder, tile tables, constants,
    and the host pre-gathered layer-0 edge rows.

    Returns (meta, per_core, consts).
    """
    import ml_dtypes
    bf16 = ml_dtypes.bfloat16
    N, E, C, B = cfg["N"], cfg["E"], cfg["C"], cfg["B"]
    NC, NBUCK = cfg["NCORES"], cfg["NBUCK"]
    SHARD, NBLK = cfg["SHARD"], cfg["NBLK"]
    SHARD_PAD = cfg["SHARD_PAD"]
    CH_ROWS, CSTART = cfg["CH_ROWS"], cfg["CSTART"]

    nf = np.asarray(inputs["node_feats"], np.float32)
    W = np.asarray(inputs["W"], np.float32)
    gamma = np.asarray(inputs["gamma"], np.float32)
    beta = np.asarray(inputs["beta"], np.float32)
    lam = np.asarray(inputs["lambdas"], np.float32)
    src = np.asarray(inputs["src"]).astype(np.int64)
    dst = np.asarray(inputs["dst"]).astype(np.int64)
    gid = np.asarray(inputs["graph_ids"]).astype(np.int64)

    deg_out = np.maximum(np.bincount(src, minlength=N).astype(np.float64), 1.0)
    deg_in = np.maximum(np.bincount(dst, minlength=N).astype(np.float64), 1.0)
    iso = (1.0 / np.sqrt(deg_out)).astype(np.float32)   # inv_sqrt_out per node
    isi = (1.0 / np.sqrt(deg_in)).astype(np.float32)    # inv_sqrt_in per node
    cnt = np.maximum(np.bincount(gid, minlength=B).astype(np.float64), 1.0)
    cnt_inv = (1.0 / cnt).astype(np.float32).reshape(B, 1)

    # softmax(lambdas) per layer, host-side (3x3 input params)
    lam64 = lam.astype(np.float64)
    ex = np.exp(lam64 - lam64.max(axis=1, keepdims=True))
    wsoft = (ex / ex.sum(axis=1, keepdims=True)).astype(np.float64)  # [L,3]

    # layer-0 message table: node_feats * inv_sqrt_out folded on host
    m0 = (nf * iso[:, None]).astype(bf16)

    # edge -> (core, block, slot, quarter-bucket, idx16)
    core = dst // SHARD
    local = dst - core * SHARD
    blk = local // 128
    slot = (local % 128).astype(np.float32)
    src_core = src // SHARD
    src_local = src - src_core * SHARD
    buck = np.digitize(src_local, CSTART[1:], right=False)
    ch_rows = np.asarray(CH_ROWS, np.int64)
    cstart = np.asarray(CSTART, np.int64)
    idx16 = (src_core * ch_rows[buck] + (src_local - cstart[buck])).astype(np.int16)

    # counts per (core, blk, buck)
    key = (core * NBLK + blk) * NBUCK + buck
    cnts = np.bincount(key, minlength=NC * NBLK * NBUCK).reshape(NC, NBLK, NBUCK)
    T = np.ceil(cnts.max(axis=0) / 128.0).astype(np.int64)  # [NBLK, NBUCK]
    # every block needs >=1 tile so PSUM gets a start matmul
    zero_blocks = T.sum(axis=1) == 0
    T[zero_blocks, 0] = 1

    TQ = T.sum(axis=0)          # tiles per bucket stream
    EQ = TQ * 128               # padded edges per stream
    # slot offset of (blk) within stream q: running sum of T[:, q]
    off_blk = np.zeros((NBLK, NBUCK), np.int64)
    off_blk[1:] = np.cumsum(T[:-1] * 128, axis=0)

    order = np.lexsort((buck, blk, core))   # sort edges by (core, blk, buck)
    per_core = []
    for k in range(NC):
        sel = order[core[order] == k]
        bblk, bbuck = blk[sel], buck[sel]
        # position within (blk, buck) group
        grp = bblk * NBUCK + bbuck
        rank = np.zeros(len(sel), np.int64)
        if len(sel):
            gcnt = np.bincount(grp, minlength=NBLK * NBUCK)
            starts = np.concatenate([[0], np.cumsum(gcnt)[:-1]])
            # edges are sorted by grp already (lexsort by (blk,buck))
            rank = np.arange(len(sel)) - starts[grp]
        pos = off_blk[bblk, bbuck] + rank           # slot within stream bbuck
        d = {}
        for q in range(NBUCK):
            eq = int(EQ[q])
            idx_q = np.zeros(eq, np.int16)
            slot_q = -np.ones(eq, np.float32)
            m = bbuck == q
            idx_q[pos[m]] = idx16[sel[m]]
            slot_q[pos[m]] = slot[sel[m]]
            d[f"idxq{q}"] = np.tile(
                np.ascontiguousarray(idx_q.reshape(-1, 16).T), (8, 1))
            d[f"slotq{q}"] = np.ascontiguousarray(
                slot_q.reshape(-1, 128).T).astype(bf16)
            # host pre-gathered layer-0 rows, laid out as the gather would:
            # stream position p -> partition p%128, tile p//128
            g0 = np.zeros((eq, C), bf16)
            g0[pos[m]] = m0[src[sel[m]]]
            d[f"gt0q{q}"] = np.ascontiguousarray(
                g0.reshape(-1, 128, C).transpose(1, 0, 2)).reshape(128, -1)
        # per-node columns for this shard (padded rows -> 1.0 / gid 0)
        lo, hi = k * SHARD, (k + 1) * SHARD
        pad = SHARD_PAD - SHARD
        isi_k = np.concatenate([isi[lo:hi], np.ones(pad, np.float32)])
        iso_k = np.concatenate([iso[lo:hi], np.ones(pad, np.float32)])
        d["inv_in_c"] = np.ascontiguousarray(isi_k.reshape(NBLK, 128).T)
        d["inv_out_c"] = np.ascontiguousarray(iso_k.reshape(NBLK, 128).T)
        gid_k = gid[lo:hi]
        G = np.zeros((SHARD_PAD, B), np.float32)
        G[np.arange(SHARD), gid_k] = 1.0
        G3 = G.reshape(NBLK, 128, B)
        d["g_oh"] = np.ascontiguousarray(
            G3.transpose(1, 0, 2)).reshape(128, NBLK * B)
        d["g_ohT"] = np.ascontiguousarray(
            G3.transpose(2, 0, 1)).reshape(B, NBLK * 128)
        per_core.append(d)

    consts = dict(
        iota=np.broadcast_to(
            np.arange(128, dtype=np.float32), (128, 128)).astype(bf16),
        wmat=np.ascontiguousarray(
            W.transpose(1, 0, 2)).reshape(C, cfg["L"] * C),
        cnt_inv=cnt_inv,
        gamma=gamma, beta=beta,
    )
    gamma_trivial = bool(np.all(gamma == 1.0) and np.all(beta == 0.0))
    meta = dict(T=T, TQ=TQ, EQ=EQ, wsoft=wsoft, gamma_trivial=gamma_trivial,
                TMAX=int(T.max()))
    return meta, per_core, consts


def build_nc(cfg, meta):
    import concourse.bacc as bacc
    import concourse.bass as bass
    import concourse.mybir as mybir
    import concourse.tile as tile

    f32 = mybir.dt.float32
    GD = f32 if cfg["GD"] == "f32" else mybir.dt.bfloat16
    C, B, L = cfg["C"], cfg["B"], cfg["L"]
    NC, NBUCK, CH, GB = cfg["NCORES"], cfg["NBUCK"], cfg["CH"], cfg["GB"]
    NBLK, SHARD_PAD = cfg["NBLK"], cfg["SHARD_PAD"]
    CH_BLOCKS, CH_ROWS, CB0 = cfg["CH_BLOCKS"], cfg["CH_ROWS"], cfg["CB0"]
    WINS = cfg["WIN"]
    EPS, SLOPE, N = cfg["EPS"], cfg["SLOPE"], cfg["N"]
    T, TQ, EQ = meta["T"], meta["TQ"], meta["EQ"]
    wsoft = meta["wsoft"]
    gtriv = meta["gamma_trivial"]
    RG = [list(range(NC))]
    eq_ = mybir.AluOpType
    AF = mybir.ActivationFunctionType

    nc = bacc.Bacc("TRN2", target_bir_lowering=False, debug=False,
                   num_devices=NC, num_swdge_queues=min(4, NBUCK))

    # ---- DRAM tensors ----
    out_t = nc.dram_tensor("out", [B, C], f32, kind="ExternalOutput")
    idx_t, slot_t, gt0_t = [], [], []
    for q in range(NBUCK):
        idx_t.append(nc.dram_tensor(f"idxq{q}", [128, int(EQ[q]) // 16],
                                    mybir.dt.int16, kind="ExternalInput"))
        slot_t.append(nc.dram_tensor(f"slotq{q}", [128, int(EQ[q]) // 128],
                                     GD, kind="ExternalInput"))
        gt0_t.append(nc.dram_tensor(f"gt0q{q}", [128, int(TQ[q]) * C],
                                    GD, kind="ExternalInput"))
    iota_t = nc.dram_tensor("iota", [128, 128], GD, kind="ExternalInput")
    invin_t = nc.dram_tensor("inv_in_c", [128, NBLK], f32, kind="ExternalInput")
    invout_t = nc.dram_tensor("inv_out_c", [128, NBLK], f32, kind="ExternalInput")
    goh_t = nc.dram_tensor("g_oh", [128, NBLK * B], f32, kind="ExternalInput")
    gohT_t = nc.dram_tensor("g_ohT", [B, NBLK * 128], f32, kind="ExternalInput")
    wmat_t = nc.dram_tensor("wmat", [C, L * C], f32, kind="ExternalInput")
    cntinv_t = nc.dram_tensor("cnt_inv", [B, 1], f32, kind="ExternalInput")
    gamma_t = nc.dram_tensor("gamma", [L, C], f32, kind="ExternalInput")
    beta_t = nc.dram_tensor("beta", [L, C], f32, kind="ExternalInput")

    # per-layer, per-quarter message tables (layers 0..L-2 feed layers 1..L-1)
    msh = [[nc.dram_tensor(f"msh{l}_{c}", [CH_ROWS[c], C], GD)
            for c in range(NBUCK)] for l in range(L - 1)]
    mfull = [[nc.dram_tensor(f"mfull{l}_{c}", [WINS[c], C], GD,
                             addr_space="Shared")
              for c in range(NBUCK)] for l in range(L - 1)]
    stin, stout = [], []
    for l in range(L):
        stin.append(nc.dram_tensor(f"stin{l}", [2 * B, C], f32))
        stout.append(nc.dram_tensor(f"stout{l}", [2 * B, C], f32,
                                    addr_space="Shared"))
    embin = nc.dram_tensor("embin", [B, C], f32)
    embout = nc.dram_tensor("embout", [B, C], f32, addr_space="Shared")

    with tile.TileContext(nc) as tc:
        with (
            tc.tile_pool(name="const", bufs=1) as cp,
            tc.tile_pool(name="big", bufs=1) as bigp,
            tc.tile_pool(name="gath", bufs=6) as gp,
            tc.tile_pool(name="work", bufs=2) as wp,
            tc.tile_pool(name="coef", bufs=1) as kp,
            tc.tile_pool(name="psum", bufs=2, space="PSUM") as pp,
            tc.tile_pool(name="psumq", bufs=2, space="PSUM") as ppq,
            tc.tile_pool(name="psum1", bufs=1, space="PSUM") as pp1,
        ):
            # ---- resident constants ----
            iota = cp.tile([128, 128], GD)
            nc.sync.dma_start(iota[:], iota_t.ap())
            wm = cp.tile([C, L, C], f32)
            nc.sync.dma_start(wm[:], wmat_t.ap().rearrange("c (l k) -> c l k", l=L))
            goh = cp.tile([128, NBLK, B], f32)
            nc.sync.dma_start(goh[:], goh_t.ap().rearrange("p (b g) -> p b g", b=NBLK))
            invin = cp.tile([128, NBLK], f32)
            nc.sync.dma_start(invin[:], invin_t.ap())
            invout = cp.tile([128, NBLK], f32)
            nc.sync.dma_start(invout[:], invout_t.ap())
            cntinv = cp.tile([B, 1], f32)
            nc.sync.dma_start(cntinv[:], cntinv_t.ap())
            ones16 = cp.tile([B, 1], f32)
            nc.vector.memset(ones16[:], 1.0)
            ones1 = cp.tile([1, B], f32)
            nc.vector.memset(ones1[:], 1.0)
            ones1p = cp.tile([1, 128], f32)
            nc.vector.memset(ones1p[:], 1.0)

            eps128 = cp.tile([128, 1], f32)
            nc.vector.memset(eps128[:], EPS)
            gam = cp.tile([L, C], f32)
            nc.sync.dma_start(gam[:], gamma_t.ap())
            bet = cp.tile([L, C], f32)
            nc.sync.dma_start(bet[:], beta_t.ap())

            hbuf = bigp.tile([128, NBLK, C], f32)
            nm_arr = cp.tile([128, NBLK], f32)
            nv_arr = cp.tile([128, NBLK], f32)

            for l in range(L):
                w0, w1, w2 = [float(x) for x in wsoft[l]]
                # ---------------- PASS A ----------------
                scopeA = nc.named_scope(f"passA_{l}"); scopeA.__enter__()
                gsx = pp1.tile([64, C], f32, tag="gsx")
                gs_p = gsx[0:B, :]
                gss_p = gsx[32:32 + B, :]
                CHL = CH   # tiles per chunk/call
                cur = [0] * NBUCK          # consumed tiles per stream
                chunks = [dict() for _ in range(NBUCK)]  # live chunk tiles
                nchunks = [(int(TQ[q]) + CHL - 1) // CHL for q in range(NBUCK)]

                IB = 8    # idx/slot load batching (chunks per DMA)
                ibatch_tiles = [None] * NBUCK
                ibatch_id = [-1] * NBUCK

                def issue_chunk(q, ci, l=l):
                    bi = ci // IB
                    if ibatch_id[q] != bi:
                        nt_b = min(IB * CHL, int(TQ[q]) - bi * IB * CHL)
                        c0 = bi * IB * CHL
                        it = None
                        if l > 0:
                            it = gp.tile([128, IB * CHL * 8], mybir.dt.int16,
                                         tag=f"i{q}", name=f"it{q}")
                            nc.sync.dma_start(
                                it[:, :nt_b * 8],
                                idx_t[q].ap()[:, c0 * 8:(c0 + nt_b) * 8])
                        st = gp.tile([128, IB * CHL], GD, tag=f"s{q}",
                                     name=f"st{q}")
                        nc.sync.dma_start(st[:, :nt_b],
                                          slot_t[q].ap()[:, c0:c0 + nt_b])
                        ibatch_tiles[q] = (it, st)
                        ibatch_id[q] = bi
                    it, st = ibatch_tiles[q]
                    r = min(CHL, int(TQ[q]) - ci * CHL)
                    co = (ci % IB) * CHL
                    gt = gp.tile([128, CHL, C], GD, tag=f"g{q}")
                    if l == 0:
                        nc.sync.dma_start(
                            gt[:, :r, :],
                            gt0_t[q].ap().rearrange("p (t c) -> p t c", c=C)
                            [:, ci * CHL:ci * CHL + r, :])
                    else:
                        nc.gpsimd.dma_gather(
                            gt[:, :r, :], mfull[l - 1][q].ap(),
                            it[:, co * 8:(co + r) * 8],
                            r * 128, r * 128, C, queue_num=q % 4)
                    S = gp.tile([128, CHL, 128], mybir.dt.float8e4,
                                tag=f"S{q}", name=f"S{q}")
                    nc.vector.tensor_tensor(
                        out=S[:, :r, :],
                        in0=iota[:].unsqueeze(1).broadcast_to([128, r, 128]),
                        in1=st[:, co:co + r].unsqueeze(2)
                            .broadcast_to([128, r, 128]),
                        op=eq_.is_equal)
                    return (gt, S)

                def get_chunk(q, ci):
                    if ci not in chunks[q]:
                        chunks[q][ci] = issue_chunk(q, ci)
                        chunks[q].pop(ci - 6, None)
                    return chunks[q][ci]

                if l > 0:
                    # front-load first chunks of streams 0-2 so their gathers
                    # flow while stream 3 waits on its AllGather chunk
                    for ci in range(4):
                        for q in range(NBUCK - 1):
                            if ci < nchunks[q]:
                                get_chunk(q, ci)

                # deferred per-block tails keep each engine's in-order queue
                # free of cross-engine head-of-line stalls:
                #   iter b: PE agg(b), ACT copy(b), PE h(b-1),
                #           ACT hbuf(b-1)+sq(b-1), PE gs/gss(b-2)
                aggTs_of = {}

                def tail_h(b):
                    aggTs = aggTs_of.pop(b)
                    h_p = pp.tile([128, C], f32, tag="hp")
                    nc.tensor.matmul(h_p[:], aggTs[:], wm[:, l, :],
                                     start=True, stop=True)
                    nc.scalar.activation(hbuf[:, b, :], h_p[:], AF.Copy,
                                         scale=invin[:, b:b + 1],
                                         accum_out=nm_arr[:, b:b + 1])
                    h2 = wp.tile([128, C], f32, tag="h2")
                    nc.scalar.activation(h2[:], hbuf[:, b, :], AF.Square,
                                         accum_out=nv_arr[:, b:b + 1])
                    return h2

                h2_of = {}

                def tail_stats(b):
                    h2 = h2_of.pop(b)
                    nc.tensor.matmul(gs_p, goh[:, b, :], hbuf[:, b, :],
                                     start=(b == 0), stop=(b == NBLK - 1))
                    nc.tensor.matmul(gss_p, goh[:, b, :], h2[:],
                                     start=(b == 0), stop=(b == NBLK - 1))

                for b in range(NBLK):
                    aggT_p = pp.tile([C, 128], f32, tag="aggT")
                    ntot = int(T[b].sum())
                    done = 0
                    for q in range(NBUCK):
                        nt = int(T[b, q])
                        t0 = cur[q]
                        cur[q] += nt
                        while nt > 0:
                            ci = t0 // CHL
                            col = t0 % CHL
                            r = min(nt, CHL - col)
                            gt, S = get_chunk(q, ci)
                            for j in range(r):
                                nc.tensor.matmul(
                                    aggT_p[:], gt[:, col + j, :], S[:, col + j, :],
                                    start=(done == 0), stop=(done == ntot - 1))
                                done += 1
                            t0 += r
                            nt -= r
                    aggTs = wp.tile([C, 128], f32, tag="aggTs")
                    nc.scalar.activation(aggTs[:], aggT_p[:], AF.Copy)
                    aggTs_of[b] = aggTs
                    if b >= 1:
                        h2_of[b - 1] = tail_h(b - 1)
                    if b >= 2:
                        tail_stats(b - 2)
                h2_of[NBLK - 1] = tail_h(NBLK - 1)
                tail_stats(NBLK - 2)
                tail_stats(NBLK - 1)

                scopeA.__exit__(None, None, None)
                scopeS = nc.named_scope(f"stats_{l}"); scopeS.__enter__()
                # ---- stats AllReduce ----
                sts_a = kp.tile([B, C], f32, tag="sts_a")
                nc.scalar.activation(sts_a[:], gs_p, AF.Copy)
                sts_b = kp.tile([B, C], f32, tag="sts_b")
                nc.scalar.activation(sts_b[:], gss_p, AF.Copy)
                nc.sync.dma_start(stin[l].ap()[0:B, :], sts_a[:])
                nc.sync.dma_start(stin[l].ap()[B:2 * B, :], sts_b[:])
                nc.gpsimd.collective_compute(
                    "AllReduce", eq_.add, ins=[stin[l].ap()],
                    outs=[stout[l].ap()], replica_groups=RG)
                # ---- per-node coefficients (local; overlap the AllReduce) ----
                nmm = kp.tile([128, NBLK], f32, tag="nmm")
                nc.vector.tensor_scalar_mul(nmm[:], nm_arr[:], 1.0 / C)
                nvm = kp.tile([128, NBLK], f32, tag="nvm")
                nc.vector.tensor_scalar_mul(nvm[:], nv_arr[:], 1.0 / C)
                nm2 = kp.tile([128, NBLK], f32, tag="nm2")
                nc.vector.tensor_tensor(out=nm2[:], in0=nmm[:], in1=nmm[:], op=eq_.mult)
                nc.vector.tensor_tensor(out=nvm[:], in0=nvm[:], in1=nm2[:], op=eq_.subtract)
                nc.scalar.activation(nvm[:], nvm[:], AF.Sqrt, bias=eps128[:])
                invn = kp.tile([128, NBLK], f32, tag="invn")
                nc.vector.reciprocal(invn[:], nvm[:])
                a_n = kp.tile([128, NBLK], f32, tag="a_n")
                nc.vector.tensor_scalar_mul(a_n[:], invn[:], w2)
                b_n = kp.tile([128, NBLK], f32, tag="b_n")
                nc.vector.tensor_tensor(out=b_n[:], in0=nmm[:],
                                        in1=a_n[:], op=eq_.mult)

                gs_t = kp.tile([B, C], f32, tag="gs_t")
                nc.sync.dma_start(gs_t[:], stout[l].ap()[0:B, :])
                gss_t = kp.tile([B, C], f32, tag="gss_t")
                nc.sync.dma_start(gss_t[:], stout[l].ap()[B:2 * B, :])
                gs, gss = gs_t[:], gss_t[:]

                # ---- coefficients A16/B16 ----
                gm = kp.tile([B, C], f32, tag="gm")
                nc.vector.tensor_scalar_mul(gm[:], gs, cntinv[:])
                gv = kp.tile([B, C], f32, tag="gv")
                nc.vector.tensor_scalar_mul(gv[:], gss, cntinv[:])
                tmp16 = kp.tile([B, C], f32, tag="tmp16")
                nc.vector.tensor_tensor(out=tmp16[:], in0=gm[:], in1=gm[:], op=eq_.mult)
                nc.vector.tensor_tensor(out=gv[:], in0=gv[:], in1=tmp16[:], op=eq_.subtract)
                nc.scalar.activation(gv[:], gv[:], AF.Sqrt, bias=eps128[0:B, :])
                igv = kp.tile([B, C], f32, tag="igv")
                nc.vector.reciprocal(igv[:], gv[:])
                bs_p = pp.tile([1, C], f32, tag="aggT")
                nc.tensor.matmul(bs_p[:], ones16[:], gs, start=True, stop=True)
                bss_p = pp.tile([1, C], f32, tag="hp")
                nc.tensor.matmul(bss_p[:], ones16[:], gss, start=True, stop=True)
                bm = kp.tile([1, C], f32, tag="bm")
                nc.vector.tensor_scalar_mul(bm[:], bs_p[:], 1.0 / N)
                bv = kp.tile([1, C], f32, tag="bv")
                nc.vector.tensor_scalar_mul(bv[:], bss_p[:], 1.0 / N)
                tmp1 = kp.tile([1, C], f32, tag="tmp1")
                nc.vector.tensor_tensor(out=tmp1[:], in0=bm[:], in1=bm[:], op=eq_.mult)
                nc.vector.tensor_tensor(out=bv[:], in0=bv[:], in1=tmp1[:], op=eq_.subtract)
                nc.scalar.activation(bv[:], bv[:], AF.Sqrt, bias=eps128[0:1, :])
                ibv = kp.tile([1, C], f32, tag="ibv")
                nc.vector.reciprocal(ibv[:], bv[:])
                # broadcast [1,C] rows to B partitions via K=1 matmul
                ibv_p = pp.tile([B, C], f32, tag="aggT")
                nc.tensor.matmul(ibv_p[:], ones1[:], ibv[:], start=True, stop=True)
                bmibv = kp.tile([1, C], f32, tag="bmibv")
                nc.vector.tensor_tensor(out=bmibv[:], in0=bm[:], in1=ibv[:], op=eq_.mult)
                bmibv_p = pp.tile([B, C], f32, tag="hp")
                nc.tensor.matmul(bmibv_p[:], ones1[:], bmibv[:], start=True, stop=True)
                A16 = kp.tile([B, C], f32, tag="A16")
                nc.vector.tensor_scalar_mul(A16[:], igv[:], w1)
                t16b = kp.tile([B, C], f32, tag="t16b")
                nc.vector.tensor_scalar_mul(t16b[:], ibv_p[:], w0)
                nc.vector.tensor_tensor(out=A16[:], in0=A16[:], in1=t16b[:], op=eq_.add)
                B16 = kp.tile([B, C], f32, tag="B16")
                nc.vector.tensor_tensor(out=B16[:], in0=gm[:], in1=igv[:], op=eq_.mult)
                nc.vector.tensor_scalar_mul(B16[:], B16[:], w1)
                nc.vector.tensor_scalar_mul(t16b[:], bmibv_p[:], w0)
                nc.vector.tensor_tensor(out=B16[:], in0=B16[:], in1=t16b[:], op=eq_.add)
                if not gtriv:
                    # broadcast gamma/beta rows to all 128 partitions for the
                    # post-norm affine (u*gamma + beta) in pass B
                    gpb_p = pp.tile([128, C], f32, tag="aggT")
                    nc.tensor.matmul(gpb_p[:], ones1p[:], gam[l:l + 1, :],
                                     start=True, stop=True)
                    gam128 = kp.tile([128, C], GD, tag="gam128")
                    nc.scalar.activation(gam128[:], gpb_p[:], AF.Copy)
                    bpb_p = pp.tile([128, C], f32, tag="hp")
                    nc.tensor.matmul(bpb_p[:], ones1p[:], bet[l:l + 1, :],
                                     start=True, stop=True)
                    bet128 = kp.tile([128, C], GD, tag="bet128")
                    nc.scalar.activation(bet128[:], bpb_p[:], AF.Copy)

                scopeS.__exit__(None, None, None)
                if l == L - 1:
                    emb_t = pp1.tile([64, C], f32, tag="gsx", name="emb_t")
                    emb_p = emb_t[0:B, :]

                # ---------------- PASS B ----------------
                scopeB = nc.named_scope(f"passB_{l}"); scopeB.__enter__()
                for c in range(NBUCK):
                    for g0 in range(CB0[c], CB0[c] + CH_BLOCKS[c], GB):
                        gb = min(GB, CB0[c] + CH_BLOCKS[c] - g0)
                        gT4 = wp.tile([B, GB * 128], f32, tag="gT4")
                        nc.sync.dma_start(
                            gT4[:, :gb * 128],
                            gohT_t.ap()[:, g0 * 128:(g0 + gb) * 128])
                        PQ = ppq.tile([128, GB, 2, C], f32, tag="PQ")
                        P_p = PQ[:, :, 0, :]
                        Q_p = PQ[:, :, 1, :]
                        for j in range(gb):
                            gT = gT4[:, j * 128:(j + 1) * 128]
                            nc.tensor.matmul(PQ[:, j, 0, :], gT, A16[:],
                                             start=True, stop=True)
                            nc.tensor.matmul(PQ[:, j, 1, :], gT, B16[:],
                                             start=True, stop=True)
                        # per-node coefficients enter as free-dim broadcast
                        # adds on DVE (psum operand read directly)
                        Pt = wp.tile([128, GB, C], f32, tag="Pt")
                        nc.vector.tensor_tensor(
                            out=Pt[:, :gb, :], in0=P_p[:, :gb, :],
                            in1=a_n[:, g0:g0 + gb].unsqueeze(2)
                                .broadcast_to([128, gb, C]), op=eq_.add)
                        u = wp.tile([128, GB, C], f32, tag="u")
                        nc.vector.tensor_tensor(
                            out=u[:, :gb, :], in0=hbuf[:, g0:g0 + gb, :],
                            in1=Pt[:, :gb, :], op=eq_.mult)
                        Qt = wp.tile([128, GB, C], f32, tag="Qt")
                        nc.vector.tensor_tensor(
                            out=Qt[:, :gb, :], in0=Q_p[:, :gb, :],
                            in1=b_n[:, g0:g0 + gb].unsqueeze(2)
                                .broadcast_to([128, gb, C]), op=eq_.add)
                        nc.vector.tensor_tensor(
                            out=u[:, :gb, :], in0=u[:, :gb, :],
                            in1=Qt[:, :gb, :], op=eq_.subtract)
                        if not gtriv:
                            nc.vector.tensor_tensor(
                                out=u[:, :gb, :], in0=u[:, :gb, :],
                                in1=gam128[:].unsqueeze(1)
                                    .broadcast_to([128, gb, C]), op=eq_.mult)
                            nc.vector.tensor_tensor(
                                out=u[:, :gb, :], in0=u[:, :gb, :],
                                in1=bet128[:].unsqueeze(1)
                                    .broadcast_to([128, gb, C]), op=eq_.add)
                        # leaky relu on DVE (ACT Lrelu ignores alpha: slope
                        # is stuck at 0.01)
                        v = wp.tile([128, GB, C], f32, tag="v")
                        nc.vector.tensor_scalar_mul(v[:, :gb, :],
                                                    u[:, :gb, :], SLOPE)
                        nc.vector.tensor_tensor(
                            out=u[:, :gb, :], in0=u[:, :gb, :],
                            in1=v[:, :gb, :], op=eq_.max)
                        if l < L - 1:
                            mt = wp.tile([128, GB, C], GD, tag="mt")
                            for j in range(gb):
                                b = g0 + j
                                nc.scalar.activation(
                                    mt[:, j, :], u[:, j, :], AF.Copy,
                                    scale=invout[:, b:b + 1])
                            r0 = (g0 - CB0[c]) * 128
                            nc.sync.dma_start(
                                msh[l][c].ap()[r0:r0 + gb * 128, :]
                                .rearrange("(j p) c -> p j c", p=128),
                                mt[:, :gb, :])
                        else:
                            for j in range(gb):
                                b = g0 + j
                                nc.tensor.matmul(
                                    emb_p, goh[:, b, :], u[:, j, :],
                                    start=(b == 0), stop=(b == NBLK - 1))
                    if l < L - 1:
                        with nc.named_scope(f"ag_{l}_{c}"):
                            nc.gpsimd.collective_compute(
                                "AllGather", eq_.bypass, ins=[msh[l][c].ap()],
                                outs=[mfull[l][c].ap()], replica_groups=RG)
                scopeB.__exit__(None, None, None)

            # ---- readout ----
            embs = kp.tile([B, C], f32, tag="embs")
            nc.scalar.activation(embs[:], emb_p, AF.Copy)
            nc.sync.dma_start(embin.ap(), embs[:])
            nc.gpsimd.collective_compute(
                "AllReduce", eq_.add, ins=[embin.ap()], outs=[embout.ap()],
                replica_groups=RG)
            embg = kp.tile([B, C], f32, tag="embg")
            nc.sync.dma_start(embg[:], embout.ap())
            nc.vector.tensor_scalar_mul(embg[:], embg[:], cntinv[:])
            embg2 = kp.tile([B, C], f32, tag="embg2")
            nc.vector.tensor_scalar_mul(embg2[:], embg[:], SLOPE)
            nc.vector.tensor_tensor(out=embg[:], in0=embg[:], in1=embg2[:], op=eq_.max)
            nc.sync.dma_start(out_t.ap(), embg[:])

    nc.compile()
    return nc


def make_in_maps(cfg, per_core, consts):
    base = dict(
        iota=consts["iota"], wmat=consts["wmat"],
        cnt_inv=consts["cnt_inv"], gamma=consts["gamma"], beta=consts["beta"],
    )
    in_maps = []
    for k in range(cfg["NCORES"]):
        d = dict(base)
        d.update(per_core[k])
        in_maps.append(d)
    return in_maps


_BUILD_CACHE = {}


def _build_cached(cfg, meta):
    key = (tuple(sorted((k, str(v)) for k, v in cfg.items())),
           meta["T"].tobytes(), meta["wsoft"].tobytes(), meta["gamma_trivial"])
    if key not in _BUILD_CACHE:
        _BUILD_CACHE[key] = build_nc(cfg, meta)
    return _BUILD_CACHE[key]


def run_cfg(cfg, inputs, trace=False, verbose=False):
    import time
    from concourse.bass_utils import run_bass_kernel_spmd
    t0 = time.time()
    cfg = _derive(cfg)
    meta, per_core, consts = prep_host(inputs, cfg)
    t1 = time.time()
    nc = _build_cached(cfg, meta)
    t2 = time.time()
    in_maps = make_in_maps(cfg, per_core, consts)
    res = run_bass_kernel_spmd(nc, in_maps, list(range(cfg["NCORES"])),
                               trace=trace)
    t3 = time.time()
    if verbose:
        print(f"[timing] prep={t1-t0:.1f}s build+compile={t2-t1:.1f}s "
              f"run={t3-t2:.1f}s", flush=True)
    return res.results[0]["out"].astype(np.float32), res


def kernel(**inputs):
    out, _ = run_cfg(_cfg_real(), inputs)
    return out


# revision 53
# speedup vs baseline: 1.2912x; 1.2912x over previous
"""Trainium2 Bass kernel for nn_CONVMGEmbedder (3-layer GraphConv + UnitedNorm + readout).

Strategy: dst-sharded graph partition over 8 NeuronCores.
- Node shard k = rows [k*12500, (k+1)*12500), padded to 12544 (98 blocks of 128).
- Edges live on their dst-owner core, grouped by (dst block, src shard-quarter),
  padded to a global (SPMD-uniform) tile table.
- Layer 0: edge rows are pre-gathered on HOST (node_feats*inv_sqrt_out taken per
  edge) and streamed sequentially via HWDGE — no random-access gather.
- Layers 1-2: dma_gather of m[src] rows from per-quarter AllGathered tables,
  one-hot S tiles (DVE iota/is_equal), PE matmuls accumulate aggT = sum_e m_e x 1_slot.
- h = (aggT.T @ W) * inv_sqrt_in (ACT copy w/ per-node scale, fused row-sum for
  node stats). All matmul operands bf16.
- UnitedNorm: u = h*P - Q with P/Q = graph coeffs (one-hot matmul) + per-node
  coeffs (K=1 matmul) accumulated in PSUM; 16KB AllReduce for graph/batch stats.
- m_{l+1} = Lrelu(u * inv_sqrt_out) via one fused ACT op, stored per shard
  quarter; per-quarter AllGathers issue progressively during pass B.
- Readout: G^T @ h3 accumulated in PSUM, AllReduce, /cnt, leaky.
"""
import math
import os
import sys

sys.path.insert(0, "/opt/trn_rl_repo")

import numpy as np


def _cfg_real():
    return dict(
        N=100000, E=1600000, C=128, B=16, L=3, NCORES=8,
        NBUCK=4, CH=8, GD="bf16", GB=2,
    )


def _derive(cfg):
    c = dict(cfg)
    c["SHARD"] = c["N"] // c["NCORES"]
    c["NBLK"] = (c["SHARD"] + 127) // 128
    c["SHARD_PAD"] = c["NBLK"] * 128
    c["NROWS"] = c["NCORES"] * c["SHARD_PAD"]
    assert c["NROWS"] % c["NBUCK"] == 0
    c["WIN"] = c["NROWS"] // c["NBUCK"]
    assert c["WIN"] <= 32768, c["WIN"]
    c["EPS"] = 1e-5
    c["SLOPE"] = 0.2
    return c


def prep_host(inputs, cfg):
    """Pure-numpy sharding prep: degrees, edge reorder, tile tables, constants,
    and the host pre-gathered layer-0 edge rows.

    Returns (meta, per_core, consts).
    """
    import ml_dtypes
    bf16 = ml_dtypes.bfloat16
    N, E, C, B = cfg["N"], cfg["E"], cfg["C"], cfg["B"]
    NC, NBUCK = cfg["NCORES"], cfg["NBUCK"]
    SHARD, NBLK = cfg["SHARD"], cfg["NBLK"]
    SHARD_PAD, WIN = cfg["SHARD_PAD"], cfg["WIN"]

    nf = np.asarray(inputs["node_feats"], np.float32)
    W = np.asarray(inputs["W"], np.float32)
    gamma = np.asarray(inputs["gamma"], np.float32)
    beta = np.asarray(inputs["beta"], np.float32)
    lam = np.asarray(inputs["lambdas"], np.float32)
    src = np.asarray(inputs["src"]).astype(np.int64)
    dst = np.asarray(inputs["dst"]).astype(np.int64)
    gid = np.asarray(inputs["graph_ids"]).astype(np.int64)

    deg_out = np.maximum(np.bincount(src, minlength=N).astype(np.float64), 1.0)
    deg_in = np.maximum(np.bincount(dst, minlength=N).astype(np.float64), 1.0)
    iso = (1.0 / np.sqrt(deg_out)).astype(np.float32)   # inv_sqrt_out per node
    isi = (1.0 / np.sqrt(deg_in)).astype(np.float32)    # inv_sqrt_in per node
    cnt = np.maximum(np.bincount(gid, minlength=B).astype(np.float64), 1.0)
    cnt_inv = (1.0 / cnt).astype(np.float32).reshape(B, 1)

    # softmax(lambdas) per layer, host-side (3x3 input params)
    lam64 = lam.astype(np.float64)
    ex = np.exp(lam64 - lam64.max(axis=1, keepdims=True))
    wsoft = (ex / ex.sum(axis=1, keepdims=True)).astype(np.float64)  # [L,3]

    # layer-0 message table: node_feats * inv_sqrt_out folded on host
    m0 = (nf * iso[:, None]).astype(bf16)

    # edge -> (core, block, slot, bucket, idx16)
    core = dst // SHARD
    local = dst - core * SHARD
    blk = local // 128
    slot = (local % 128).astype(np.float32)
    row = (src // SHARD) * SHARD_PAD + (src % SHARD)   # padded table row
    buck = row // WIN
    idx16 = (row % WIN).astype(np.int16)

    # counts per (core, blk, buck)
    key = (core * NBLK + blk) * NBUCK + buck
    cnts = np.bincount(key, minlength=NC * NBLK * NBUCK).reshape(NC, NBLK, NBUCK)
    T = np.ceil(cnts.max(axis=0) / 128.0).astype(np.int64)  # [NBLK, NBUCK]
    # every block needs >=1 tile so PSUM gets a start matmul
    zero_blocks = T.sum(axis=1) == 0
    T[zero_blocks, 0] = 1

    TQ = T.sum(axis=0)          # tiles per bucket stream
    EQ = TQ * 128               # padded edges per stream
    # slot offset of (blk) within stream q: running sum of T[:, q]
    off_blk = np.zeros((NBLK, NBUCK), np.int64)
    off_blk[1:] = np.cumsum(T[:-1] * 128, axis=0)

    order = np.lexsort((buck, blk, core))   # sort edges by (core, blk, buck)
    per_core = []
    for k in range(NC):
        sel = order[core[order] == k]
        bblk, bbuck = blk[sel], buck[sel]
        # position within (blk, buck) group
        grp = bblk * NBUCK + bbuck
        rank = np.zeros(len(sel), np.int64)
        if len(sel):
            gcnt = np.bincount(grp, minlength=NBLK * NBUCK)
            starts = np.concatenate([[0], np.cumsum(gcnt)[:-1]])
            # edges are sorted by grp already (lexsort by (blk,buck))
            rank = np.arange(len(sel)) - starts[grp]
        pos = off_blk[bblk, bbuck] + rank           # slot within stream bbuck
        d = {}
        for q in range(NBUCK):
            eq = int(EQ[q])
            idx_q = np.zeros(eq, np.int16)
            slot_q = -np.ones(eq, np.float32)
            m = bbuck == q
            idx_q[pos[m]] = idx16[sel[m]]
            slot_q[pos[m]] = slot[sel[m]]
            d[f"idxq{q}"] = np.tile(
                np.ascontiguousarray(idx_q.reshape(-1, 16).T), (8, 1))
            d[f"slotq{q}"] = np.ascontiguousarray(
                slot_q.reshape(-1, 128).T).astype(bf16)
            # host pre-gathered layer-0 rows, laid out as the gather would:
            # stream position p -> partition p%128, tile p//128
            g0 = np.zeros((eq, C), bf16)
            g0[pos[m]] = m0[src[sel[m]]]
            d[f"gt0q{q}"] = np.ascontiguousarray(
                g0.reshape(-1, 128, C).transpose(1, 0, 2)).reshape(128, -1)
        # per-node columns for this shard (padded rows -> 1.0 / gid 0)
        lo, hi = k * SHARD, (k + 1) * SHARD
        pad = SHARD_PAD - SHARD
        isi_k = np.concatenate([isi[lo:hi], np.ones(pad, np.float32)])
        iso_k = np.concatenate([iso[lo:hi], np.ones(pad, np.float32)])
        d["inv_in_c"] = np.ascontiguousarray(isi_k.reshape(NBLK, 128).T)
        d["inv_out_c"] = np.ascontiguousarray(iso_k.reshape(NBLK, 128).T)
        gid_k = gid[lo:hi]
        G = np.zeros((SHARD_PAD, B), np.float32)
        G[np.arange(SHARD), gid_k] = 1.0
        G3 = G.reshape(NBLK, 128, B)
        d["g_oh"] = np.ascontiguousarray(
            G3.transpose(1, 0, 2)).reshape(128, NBLK * B)
        d["g_ohT"] = np.ascontiguousarray(
            G3.transpose(2, 0, 1)).reshape(B, NBLK * 128)
        per_core.append(d)

    consts = dict(
        iota=np.broadcast_to(
            np.arange(128, dtype=np.float32), (128, 128)).astype(bf16),
        wmat=np.ascontiguousarray(
            W.transpose(1, 0, 2)).reshape(C, cfg["L"] * C),
        cnt_inv=cnt_inv,
        gamma=gamma, beta=beta,
    )
    gamma_trivial = bool(np.all(gamma == 1.0) and np.all(beta == 0.0))
    meta = dict(T=T, TQ=TQ, EQ=EQ, wsoft=wsoft, gamma_trivial=gamma_trivial,
                TMAX=int(T.max()))
    return meta, per_core, consts


def build_nc(cfg, meta):
    import concourse.bacc as bacc
    import concourse.bass as bass
    import concourse.mybir as mybir
    import concourse.tile as tile

    f32 = mybir.dt.float32
    GD = f32 if cfg["GD"] == "f32" else mybir.dt.bfloat16
    C, B, L = cfg["C"], cfg["B"], cfg["L"]
    NC, NBUCK, CH, GB = cfg["NCORES"], cfg["NBUCK"], cfg["CH"], cfg["GB"]
    NBLK, SHARD_PAD = cfg["NBLK"], cfg["SHARD_PAD"]
    NROWS, WIN = cfg["NROWS"], cfg["WIN"]
    EPS, SLOPE, N = cfg["EPS"], cfg["SLOPE"], cfg["N"]
    T, TQ, EQ = meta["T"], meta["TQ"], meta["EQ"]
    wsoft = meta["wsoft"]
    gtriv = meta["gamma_trivial"]
    RG = [list(range(NC))]
    eq_ = mybir.AluOpType
    AF = mybir.ActivationFunctionType

    nc = bacc.Bacc("TRN2", target_bir_lowering=False, debug=False,
                   num_devices=NC, num_swdge_queues=min(4, NBUCK))

    # ---- DRAM tensors ----
    out_t = nc.dram_tensor("out", [B, C], f32, kind="ExternalOutput")
    idx_t, slot_t, gt0_t = [], [], []
    for q in range(NBUCK):
        idx_t.append(nc.dram_tensor(f"idxq{q}", [128, int(EQ[q]) // 16],
                                    mybir.dt.int16, kind="ExternalInput"))
        slot_t.append(nc.dram_tensor(f"slotq{q}", [128, int(EQ[q]) // 128],
                                     GD, kind="ExternalInput"))
        gt0_t.append(nc.dram_tensor(f"gt0q{q}", [128, int(TQ[q]) * C],
                                    GD, kind="ExternalInput"))
    iota_t = nc.dram_tensor("iota", [128, 128], GD, kind="ExternalInput")
    invin_t = nc.dram_tensor("inv_in_c", [128, NBLK], f32, kind="ExternalInput")
    invout_t = nc.dram_tensor("inv_out_c", [128, NBLK], f32, kind="ExternalInput")
    goh_t = nc.dram_tensor("g_oh", [128, NBLK * B], f32, kind="ExternalInput")
    gohT_t = nc.dram_tensor("g_ohT", [B, NBLK * 128], f32, kind="ExternalInput")
    wmat_t = nc.dram_tensor("wmat", [C, L * C], f32, kind="ExternalInput")
    cntinv_t = nc.dram_tensor("cnt_inv", [B, 1], f32, kind="ExternalInput")
    gamma_t = nc.dram_tensor("gamma", [L, C], f32, kind="ExternalInput")
    beta_t = nc.dram_tensor("beta", [L, C], f32, kind="ExternalInput")

    # per-layer message tables (layers 0..L-2 feed layers 1..L-1)
    msh = [nc.dram_tensor(f"msh{l}", [SHARD_PAD, C], GD)
           for l in range(L - 1)]
    mfull = [nc.dram_tensor(f"mfull{l}", [NROWS, C], GD, addr_space="Shared")
             for l in range(L - 1)]
    stin, stout = [], []
    for l in range(L):
        stin.append(nc.dram_tensor(f"stin{l}", [2 * B, C], f32))
        stout.append(nc.dram_tensor(f"stout{l}", [2 * B, C], f32,
                                    addr_space="Shared"))
    embin = nc.dram_tensor("embin", [B, C], f32)
    embout = nc.dram_tensor("embout", [B, C], f32, addr_space="Shared")

    with tile.TileContext(nc) as tc:
        with (
            tc.tile_pool(name="const", bufs=1) as cp,
            tc.tile_pool(name="big", bufs=1) as bigp,
            tc.tile_pool(name="gath", bufs=6) as gp,
            tc.tile_pool(name="work", bufs=2) as wp,
            tc.tile_pool(name="coef", bufs=1) as kp,
            tc.tile_pool(name="psum", bufs=2, space="PSUM") as pp,
            tc.tile_pool(name="psumq", bufs=1, space="PSUM") as ppq,
            tc.tile_pool(name="psum1", bufs=1, space="PSUM") as pp1,
        ):
            # ---- resident constants ----
            iota = cp.tile([128, 128], GD)
            nc.sync.dma_start(iota[:], iota_t.ap())
            wm = cp.tile([C, L, C], f32)
            nc.sync.dma_start(wm[:], wmat_t.ap().rearrange("c (l k) -> c l k", l=L))
            goh = cp.tile([128, NBLK, B], f32)
            nc.sync.dma_start(goh[:], goh_t.ap().rearrange("p (b g) -> p b g", b=NBLK))
            invin = cp.tile([128, NBLK], f32)
            nc.sync.dma_start(invin[:], invin_t.ap())
            invout = cp.tile([128, NBLK], f32)
            nc.sync.dma_start(invout[:], invout_t.ap())
            cntinv = cp.tile([B, 1], f32)
            nc.sync.dma_start(cntinv[:], cntinv_t.ap())
            ones16 = cp.tile([B, 1], f32)
            nc.vector.memset(ones16[:], 1.0)
            ones1 = cp.tile([1, B], f32)
            nc.vector.memset(ones1[:], 1.0)
            ones1p = cp.tile([1, 128], f32)
            nc.vector.memset(ones1p[:], 1.0)

            eps128 = cp.tile([128, 1], f32)
            nc.vector.memset(eps128[:], EPS)
            gam = cp.tile([L, C], f32)
            nc.sync.dma_start(gam[:], gamma_t.ap())
            bet = cp.tile([L, C], f32)
            nc.sync.dma_start(bet[:], beta_t.ap())

            hbuf = bigp.tile([128, NBLK, C], f32)
            nm_arr = cp.tile([128, NBLK], f32)
            nv_arr = cp.tile([128, NBLK], f32)

            for l in range(L):
                w0, w1, w2 = [float(x) for x in wsoft[l]]
                # ---------------- PASS A ----------------
                scopeA = nc.named_scope(f"passA_{l}"); scopeA.__enter__()
                gsx = pp1.tile([64, C], f32, tag="gsx")
                gs_p = gsx[0:B, :]
                gss_p = gsx[32:32 + B, :]
                CHL = CH   # tiles per chunk/call
                cur = [0] * NBUCK          # consumed tiles per stream
                chunks = [dict() for _ in range(NBUCK)]  # live chunk tiles
                nchunks = [(int(TQ[q]) + CHL - 1) // CHL for q in range(NBUCK)]

                IB = 8    # idx/slot load batching (chunks per DMA)
                ibatch_tiles = [None] * NBUCK
                ibatch_id = [-1] * NBUCK

                def issue_chunk(q, ci, l=l):
                    bi = ci // IB
                    if ibatch_id[q] != bi:
                        nt_b = min(IB * CHL, int(TQ[q]) - bi * IB * CHL)
                        c0 = bi * IB * CHL
                        it = None
                        if l > 0:
                            it = gp.tile([128, IB * CHL * 8], mybir.dt.int16,
                                         tag=f"i{q}", name=f"it{q}")
                            nc.sync.dma_start(
                                it[:, :nt_b * 8],
                                idx_t[q].ap()[:, c0 * 8:(c0 + nt_b) * 8])
                        st = gp.tile([128, IB * CHL], GD, tag=f"s{q}",
                                     name=f"st{q}")
                        nc.sync.dma_start(st[:, :nt_b],
                                          slot_t[q].ap()[:, c0:c0 + nt_b])
                        ibatch_tiles[q] = (it, st)
                        ibatch_id[q] = bi
                    it, st = ibatch_tiles[q]
                    r = min(CHL, int(TQ[q]) - ci * CHL)
                    co = (ci % IB) * CHL
                    gt = gp.tile([128, CHL, C], GD, tag=f"g{q}")
                    if l == 0:
                        nc.sync.dma_start(
                            gt[:, :r, :],
                            gt0_t[q].ap().rearrange("p (t c) -> p t c", c=C)
                            [:, ci * CHL:ci * CHL + r, :])
                    else:
                        nc.gpsimd.dma_gather(
                            gt[:, :r, :],
                            mfull[l - 1].ap()[q * WIN:(q + 1) * WIN, :],
                            it[:, co * 8:(co + r) * 8],
                            r * 128, r * 128, C, queue_num=q % 4)
                    S = gp.tile([128, CHL, 128], mybir.dt.float8e4,
                                tag=f"S{q}", name=f"S{q}")
                    nc.vector.tensor_tensor(
                        out=S[:, :r, :],
                        in0=iota[:].unsqueeze(1).broadcast_to([128, r, 128]),
                        in1=st[:, co:co + r].unsqueeze(2)
                            .broadcast_to([128, r, 128]),
                        op=eq_.is_equal)
                    return (gt, S)

                def get_chunk(q, ci):
                    if ci not in chunks[q]:
                        chunks[q][ci] = issue_chunk(q, ci)
                        chunks[q].pop(ci - 6, None)
                    return chunks[q][ci]

                if l > 0:
                    # front-load first chunks of every stream: the engine can
                    # start all four queues the moment the AllGather lands
                    for ci in range(3):
                        for q in range(NBUCK):
                            if ci < nchunks[q]:
                                get_chunk(q, ci)

                # deferred per-block tails keep each engine's in-order queue
                # free of cross-engine head-of-line stalls:
                #   iter b: PE agg(b), ACT copy(b), PE h(b-1),
                #           ACT hbuf(b-1)+sq(b-1), PE gs/gss(b-2)
                aggTs_of = {}

                def tail_h(b):
                    aggTs = aggTs_of.pop(b)
                    h_p = pp.tile([128, C], f32, tag="hp")
                    nc.tensor.matmul(h_p[:], aggTs[:], wm[:, l, :],
                                     start=True, stop=True)
                    nc.scalar.activation(hbuf[:, b, :], h_p[:], AF.Copy,
                                         scale=invin[:, b:b + 1],
                                         accum_out=nm_arr[:, b:b + 1])
                    h2 = wp.tile([128, C], f32, tag="h2")
                    nc.scalar.activation(h2[:], hbuf[:, b, :], AF.Square,
                                         accum_out=nv_arr[:, b:b + 1])
                    return h2

                h2_of = {}

                def tail_stats(b):
                    h2 = h2_of.pop(b)
                    nc.tensor.matmul(gs_p, goh[:, b, :], hbuf[:, b, :],
                                     start=(b == 0), stop=(b == NBLK - 1))
                    nc.tensor.matmul(gss_p, goh[:, b, :], h2[:],
                                     start=(b == 0), stop=(b == NBLK - 1))

                for b in range(NBLK):
                    aggT_p = pp.tile([C, 128], f32, tag="aggT")
                    ntot = int(T[b].sum())
                    done = 0
                    for q in range(NBUCK):
                        nt = int(T[b, q])
                        t0 = cur[q]
                        cur[q] += nt
                        while nt > 0:
                            ci = t0 // CHL
                            col = t0 % CHL
                            r = min(nt, CHL - col)
                            gt, S = get_chunk(q, ci)
                            for j in range(r):
                                nc.tensor.matmul(
                                    aggT_p[:], gt[:, col + j, :], S[:, col + j, :],
                                    start=(done == 0), stop=(done == ntot - 1))
                                done += 1
                            t0 += r
                            nt -= r
                    aggTs = wp.tile([C, 128], f32, tag="aggTs")
                    nc.scalar.activation(aggTs[:], aggT_p[:], AF.Copy)
                    aggTs_of[b] = aggTs
                    if b >= 1:
                        h2_of[b - 1] = tail_h(b - 1)
                    if b >= 2:
                        tail_stats(b - 2)
                h2_of[NBLK - 1] = tail_h(NBLK - 1)
                tail_stats(NBLK - 2)
                tail_stats(NBLK - 1)

                scopeA.__exit__(None, None, None)
                scopeS = nc.named_scope(f"stats_{l}"); scopeS.__enter__()
                # ---- stats AllReduce ----
                sts_a = kp.tile([B, C], f32, tag="sts_a")
                nc.scalar.activation(sts_a[:], gs_p, AF.Copy)
                sts_b = kp.tile([B, C], f32, tag="sts_b")
                nc.scalar.activation(sts_b[:], gss_p, AF.Copy)
                nc.sync.dma_start(stin[l].ap()[0:B, :], sts_a[:])
                nc.sync.dma_start(stin[l].ap()[B:2 * B, :], sts_b[:])
                nc.gpsimd.collective_compute(
                    "AllReduce", eq_.add, ins=[stin[l].ap()],
                    outs=[stout[l].ap()], replica_groups=RG)
                # ---- per-node coefficients (local; overlap the AllReduce) ----
                nmm = kp.tile([128, NBLK], f32, tag="nmm")
                nc.vector.tensor_scalar_mul(nmm[:], nm_arr[:], 1.0 / C)
                nvm = kp.tile([128, NBLK], f32, tag="nvm")
                nc.vector.tensor_scalar_mul(nvm[:], nv_arr[:], 1.0 / C)
                nm2 = kp.tile([128, NBLK], f32, tag="nm2")
                nc.vector.tensor_tensor(out=nm2[:], in0=nmm[:], in1=nmm[:], op=eq_.mult)
                nc.vector.tensor_tensor(out=nvm[:], in0=nvm[:], in1=nm2[:], op=eq_.subtract)
                nc.scalar.activation(nvm[:], nvm[:], AF.Sqrt, bias=eps128[:])
                invn = kp.tile([128, NBLK], f32, tag="invn")
                nc.vector.reciprocal(invn[:], nvm[:])
                a_n = kp.tile([128, NBLK], f32, tag="a_n")
                nc.vector.tensor_scalar_mul(a_n[:], invn[:], w2)
                b_n = kp.tile([128, NBLK], f32, tag="b_n")
                nc.vector.tensor_tensor(out=b_n[:], in0=nmm[:],
                                        in1=a_n[:], op=eq_.mult)

                gs_t = kp.tile([B, C], f32, tag="gs_t")
                nc.sync.dma_start(gs_t[:], stout[l].ap()[0:B, :])
                gss_t = kp.tile([B, C], f32, tag="gss_t")
                nc.sync.dma_start(gss_t[:], stout[l].ap()[B:2 * B, :])
                gs, gss = gs_t[:], gss_t[:]

                # ---- coefficients A16/B16 ----
                gm = kp.tile([B, C], f32, tag="gm")
                nc.vector.tensor_scalar_mul(gm[:], gs, cntinv[:])
                gv = kp.tile([B, C], f32, tag="gv")
                nc.vector.tensor_scalar_mul(gv[:], gss, cntinv[:])
                tmp16 = kp.tile([B, C], f32, tag="tmp16")
                nc.vector.tensor_tensor(out=tmp16[:], in0=gm[:], in1=gm[:], op=eq_.mult)
                nc.vector.tensor_tensor(out=gv[:], in0=gv[:], in1=tmp16[:], op=eq_.subtract)
                nc.scalar.activation(gv[:], gv[:], AF.Sqrt, bias=eps128[0:B, :])
                igv = kp.tile([B, C], f32, tag="igv")
                nc.vector.reciprocal(igv[:], gv[:])
                bs_p = pp.tile([1, C], f32, tag="aggT")
                nc.tensor.matmul(bs_p[:], ones16[:], gs, start=True, stop=True)
                bss_p = pp.tile([1, C], f32, tag="hp")
                nc.tensor.matmul(bss_p[:], ones16[:], gss, start=True, stop=True)
                bm = kp.tile([1, C], f32, tag="bm")
                nc.vector.tensor_scalar_mul(bm[:], bs_p[:], 1.0 / N)
                bv = kp.tile([1, C], f32, tag="bv")
                nc.vector.tensor_scalar_mul(bv[:], bss_p[:], 1.0 / N)
                tmp1 = kp.tile([1, C], f32, tag="tmp1")
                nc.vector.tensor_tensor(out=tmp1[:], in0=bm[:], in1=bm[:], op=eq_.mult)
                nc.vector.tensor_tensor(out=bv[:], in0=bv[:], in1=tmp1[:], op=eq_.subtract)
                nc.scalar.activation(bv[:], bv[:], AF.Sqrt, bias=eps128[0:1, :])
                ibv = kp.tile([1, C], f32, tag="ibv")
                nc.vector.reciprocal(ibv[:], bv[:])
                # broadcast [1,C] rows to B partitions via K=1 matmul
                ibv_p = pp.tile([B, C], f32, tag="aggT")
                nc.tensor.matmul(ibv_p[:], ones1[:], ibv[:], start=True, stop=True)
                bmibv = kp.tile([1, C], f32, tag="bmibv")
                nc.vector.tensor_tensor(out=bmibv[:], in0=bm[:], in1=ibv[:], op=eq_.mult)
                bmibv_p = pp.tile([B, C], f32, tag="hp")
                nc.tensor.matmul(bmibv_p[:], ones1[:], bmibv[:], start=True, stop=True)
                A16 = kp.tile([B, C], f32, tag="A16")
                nc.vector.tensor_scalar_mul(A16[:], igv[:], w1)
                t16b = kp.tile([B, C], f32, tag="t16b")
                nc.vector.tensor_scalar_mul(t16b[:], ibv_p[:], w0)
                nc.vector.tensor_tensor(out=A16[:], in0=A16[:], in1=t16b[:], op=eq_.add)
                B16 = kp.tile([B, C], f32, tag="B16")
                nc.vector.tensor_tensor(out=B16[:], in0=gm[:], in1=igv[:], op=eq_.mult)
                nc.vector.tensor_scalar_mul(B16[:], B16[:], w1)
                nc.vector.tensor_scalar_mul(t16b[:], bmibv_p[:], w0)
                nc.vector.tensor_tensor(out=B16[:], in0=B16[:], in1=t16b[:], op=eq_.add)
                if not gtriv:
                    # broadcast gamma/beta rows to all 128 partitions for the
                    # post-norm affine (u*gamma + beta) in pass B
                    gpb_p = pp.tile([128, C], f32, tag="aggT")
                    nc.tensor.matmul(gpb_p[:], ones1p[:], gam[l:l + 1, :],
                                     start=True, stop=True)
                    gam128 = kp.tile([128, C], GD, tag="gam128")
                    nc.scalar.activation(gam128[:], gpb_p[:], AF.Copy)
                    bpb_p = pp.tile([128, C], f32, tag="hp")
                    nc.tensor.matmul(bpb_p[:], ones1p[:], bet[l:l + 1, :],
                                     start=True, stop=True)
                    bet128 = kp.tile([128, C], GD, tag="bet128")
                    nc.scalar.activation(bet128[:], bpb_p[:], AF.Copy)

                scopeS.__exit__(None, None, None)
                if l == L - 1:
                    emb_t = pp1.tile([64, C], f32, tag="gsx", name="emb_t")
                    emb_p = emb_t[0:B, :]

                # ---------------- PASS B ----------------
                scopeB = nc.named_scope(f"passB_{l}"); scopeB.__enter__()
                for g0 in range(0, NBLK, GB):
                    gb = min(GB, NBLK - g0)
                    gT4 = wp.tile([B, GB * 128], f32, tag="gT4")
                    nc.sync.dma_start(
                        gT4[:, :gb * 128],
                        gohT_t.ap()[:, g0 * 128:(g0 + gb) * 128])
                    P_p = ppq.tile([128, GB, C], f32, tag="Pp")
                    Q_p = ppq.tile([128, GB, C], f32, tag="Qp")
                    for j in range(gb):
                        gT = gT4[:, j * 128:(j + 1) * 128]
                        nc.tensor.matmul(P_p[:, j, :], gT, A16[:],
                                         start=True, stop=True)
                        nc.tensor.matmul(Q_p[:, j, :], gT, B16[:],
                                         start=True, stop=True)
                    # per-node coefficients enter as free-dim broadcast
                    # adds on DVE (psum operand read directly)
                    Pt = wp.tile([128, GB, C], f32, tag="Pt")
                    nc.vector.tensor_tensor(
                        out=Pt[:, :gb, :], in0=P_p[:, :gb, :],
                        in1=a_n[:, g0:g0 + gb].unsqueeze(2)
                            .broadcast_to([128, gb, C]), op=eq_.add)
                    u = wp.tile([128, GB, C], f32, tag="u")
                    nc.vector.tensor_tensor(
                        out=u[:, :gb, :], in0=hbuf[:, g0:g0 + gb, :],
                        in1=Pt[:, :gb, :], op=eq_.mult)
                    Qt = wp.tile([128, GB, C], f32, tag="Qt")
                    nc.vector.tensor_tensor(
                        out=Qt[:, :gb, :], in0=Q_p[:, :gb, :],
                        in1=b_n[:, g0:g0 + gb].unsqueeze(2)
                            .broadcast_to([128, gb, C]), op=eq_.add)
                    nc.vector.tensor_tensor(
                        out=u[:, :gb, :], in0=u[:, :gb, :],
                        in1=Qt[:, :gb, :], op=eq_.subtract)
                    if not gtriv:
                        nc.vector.tensor_tensor(
                            out=u[:, :gb, :], in0=u[:, :gb, :],
                            in1=gam128[:].unsqueeze(1)
                                .broadcast_to([128, gb, C]), op=eq_.mult)
                        nc.vector.tensor_tensor(
                            out=u[:, :gb, :], in0=u[:, :gb, :],
                            in1=bet128[:].unsqueeze(1)
                                .broadcast_to([128, gb, C]), op=eq_.add)
                    # leaky relu on DVE (ACT Lrelu ignores alpha: slope
                    # is stuck at 0.01)
                    v = wp.tile([128, GB, C], f32, tag="v")
                    nc.vector.tensor_scalar_mul(v[:, :gb, :],
                                                u[:, :gb, :], SLOPE)
                    nc.vector.tensor_tensor(
                        out=u[:, :gb, :], in0=u[:, :gb, :],
                        in1=v[:, :gb, :], op=eq_.max)
                    if l < L - 1:
                        # next-layer table rows: m = leaky(u) * inv_sqrt_out,
                        # degree scale as one DVE broadcast op per group
                        mt = wp.tile([128, GB, C], GD, tag="mt")
                        nc.vector.tensor_tensor(
                            out=mt[:, :gb, :], in0=u[:, :gb, :],
                            in1=invout[:, g0:g0 + gb].unsqueeze(2)
                                .broadcast_to([128, gb, C]), op=eq_.mult)
                        nc.sync.dma_start(
                            msh[l].ap()[g0 * 128:(g0 + gb) * 128, :]
                            .rearrange("(j p) c -> p j c", p=128),
                            mt[:, :gb, :])
                    else:
                        for j in range(gb):
                            b = g0 + j
                            nc.tensor.matmul(
                                emb_p, goh[:, b, :], u[:, j, :],
                                start=(b == 0), stop=(b == NBLK - 1))
                if l < L - 1:
                    with nc.named_scope(f"ag_{l}"):
                        nc.gpsimd.collective_compute(
                            "AllGather", eq_.bypass, ins=[msh[l].ap()],
                            outs=[mfull[l].ap()], replica_groups=RG)
                scopeB.__exit__(None, None, None)

            # ---- readout ----
            embs = kp.tile([B, C], f32, tag="embs")
            nc.scalar.activation(embs[:], emb_p, AF.Copy)
            nc.sync.dma_start(embin.ap(), embs[:])
            nc.gpsimd.collective_compute(
                "AllReduce", eq_.add, ins=[embin.ap()], outs=[embout.ap()],
                replica_groups=RG)
            embg = kp.tile([B, C], f32, tag="embg")
            nc.sync.dma_start(embg[:], embout.ap())
            nc.vector.tensor_scalar_mul(embg[:], embg[:], cntinv[:])
            embg2 = kp.tile([B, C], f32, tag="embg2")
            nc.vector.tensor_scalar_mul(embg2[:], embg[:], SLOPE)
            nc.vector.tensor_tensor(out=embg[:], in0=embg[:], in1=embg2[:], op=eq_.max)
            nc.sync.dma_start(out_t.ap(), embg[:])

    nc.compile()
    return nc


def make_in_maps(cfg, per_core, consts):
    base = dict(
        iota=consts["iota"], wmat=consts["wmat"],
        cnt_inv=consts["cnt_inv"], gamma=consts["gamma"], beta=consts["beta"],
    )
    in_maps = []
    for k in range(cfg["NCORES"]):
        d = dict(base)
        d.update(per_core[k])
        in_maps.append(d)
    return in_maps


_BUILD_CACHE = {}


def _build_cached(cfg, meta):
    key = (tuple(sorted((k, str(v)) for k, v in cfg.items())),
           meta["T"].tobytes(), meta["wsoft"].tobytes(), meta["gamma_trivial"])
    if key not in _BUILD_CACHE:
        _BUILD_CACHE[key] = build_nc(cfg, meta)
    return _BUILD_CACHE[key]


def run_cfg(cfg, inputs, trace=False, verbose=False):
    import time
    from concourse.bass_utils import run_bass_kernel_spmd
    t0 = time.time()
    cfg = _derive(cfg)
    meta, per_core, consts = prep_host(inputs, cfg)
    t1 = time.time()
    nc = _build_cached(cfg, meta)
    t2 = time.time()
    in_maps = make_in_maps(cfg, per_core, consts)
    res = run_bass_kernel_spmd(nc, in_maps, list(range(cfg["NCORES"])),
                               trace=trace)
    t3 = time.time()
    if verbose:
        print(f"[timing] prep={t1-t0:.1f}s build+compile={t2-t1:.1f}s "
              f"run={t3-t2:.1f}s", flush=True)
    return res.results[0]["out"].astype(np.float32), res


def kernel(**inputs):
    out, _ = run_cfg(_cfg_real(), inputs)
    return out
